# revision 20
# baseline (speedup 1.0000x reference)
"""Trainium2 Bass kernel for DetectionLayer (refine + per-class NMS).

Contract: kernel(rois, probs, deltas) with FULL inputs
  rois   [16, 4096, 4]   f32
  probs  [16, 4096, 81]  f32
  deltas [16, 4096, 81, 4] f32
returns [16, 100, 6] f32 detections, matching the jax reference.

Sharding: pure data parallel - 2 images per core across 8 NeuronCores.

The always-taken fast path (probs stream + confidence gate + early exit)
is written in raw Bass with explicit semaphores so it starts before and
ends without TileContext's entry/exit handshakes: probs stream over the
Sync HWDGE queue in 7 decreasing chunks at ~420GB/s, DVE is_ge+count and
ACT sign+accumulate gate alternate chunks as they land, a PE ones-matmul
folds partitions, and one int32 reduce feeds the branch.  The all-zeros
output is DMA'd to HBM up front (hidden under the probs window), so when
no prob reaches min-confidence the kernel ends right after the branch.
The cold path (refine + per-class NMS, only when any prob >= 0.7) lives
in a TileContext nested inside the raw nc.If body, so its scheduling
cost is paid only when detections exist.
"""

import os as _os

import numpy as np

import concourse.bacc as bacc
import concourse.bass as bass
import concourse.bass_isa as bass_isa
import concourse.mybir as mybir
from concourse.tile import TileContext

B = 16
NCORES = 8
BPC = B // NCORES
N = 4096
C = 81
K = 100
P = 128
NP = N // P
NEG = -1e9
MIN_CONF = 0.7
NMS_T = 0.3
F32 = mybir.dt.float32
I32 = mybir.dt.int32

NOGUARD = _os.environ.get("DETK_NOGUARD", "0") == "1"

# (img, lo, hi, eng): eng 0 = DVE is_ge, 1 = ACT sign.  All on the Sync
# HWDGE queue (the Act queue is deprioritized by the DMA engines), sizes
# decreasing, engines alternating, last chunk tiny + DVE.
CHUNKS = [
    (0, 0, 18, 0),
    (0, 18, 32, 1),
    (1, 0, 11, 0),
    (1, 11, 19, 1),
    (1, 19, 27, 0),
    (1, 27, 30, 1),
    (1, 30, 32, 0),
]
if _os.environ.get("DETK_CHUNKS"):
    import json as _json
    CHUNKS = [tuple(c) for c in _json.loads(_os.environ["DETK_CHUNKS"])]

_DVE_N = [hi - lo for (_, lo, hi, e) in CHUNKS if e == 0]
_ACT_N = [hi - lo for (_, lo, hi, e) in CHUNKS if e == 1]


def _refine_twin(nc, tc, sm, ptw, scw, rt, dt_, crev, state):
    """Cold path, both images at once (see kernel.py)."""
    NT = BPC * NP
    pt = ptw
    scores = scw.rearrange("p b n -> p (b n)")
    rtf = rt.rearrange("p b n k -> p (b n) k")

    nc.vector.reduce_max(scores, pt, axis=mybir.AxisListType.X)
    ge = sm.tile([P, NT], F32, tag="ge")
    nc.vector.tensor_single_scalar(ge, scores, MIN_CONF,
                                   op=mybir.AluOpType.is_ge)

    m = pt
    nc.vector.tensor_tensor(
        m, pt, scores.unsqueeze(2).to_broadcast([P, NT, C]),
        op=mybir.AluOpType.is_equal,
    )

    d_perm = dt_.rearrange("p b n c k -> p (b n) k c")
    nc.vector.tensor_tensor(
        d_perm, d_perm, m.unsqueeze(2).to_broadcast([P, NT, 4, C]),
        op=mybir.AluOpType.mult,
    )
    dsel = sm.tile([P, NT, 4], F32, tag="dsel")
    nc.vector.reduce_sum(dsel, d_perm, axis=mybir.AxisListType.X)

    nc.vector.tensor_tensor(m, m, crev, op=mybir.AluOpType.mult)
    cid = sm.tile([P, NT], F32, tag="cid")
    nc.vector.reduce_max(cid, m, axis=mybir.AxisListType.X)
    nc.vector.tensor_scalar(
        out=cid, in0=cid, scalar1=-1.0, scalar2=float(C - 1),
        op0=mybir.AluOpType.mult, op1=mybir.AluOpType.add,
    )

    nc.vector.tensor_scalar_mul(dsel[:, :, 0:2], dsel[:, :, 0:2], 0.1)
    nc.vector.tensor_scalar_mul(dsel[:, :, 2:4], dsel[:, :, 2:4], 0.2)

    h = sm.tile([P, NT], F32, tag="h")
    w = sm.tile([P, NT], F32, tag="w")
    nc.vector.tensor_sub(h, rtf[:, :, 2], rtf[:, :, 0])
    nc.vector.tensor_sub(w, rtf[:, :, 3], rtf[:, :, 1])
    t1 = sm.tile([P, NT], F32, tag="t1")
    t2 = sm.tile([P, NT], F32, tag="t2")
    cy = sm.tile([P, NT], F32, tag="cy")
    cx = sm.tile([P, NT], F32, tag="cx")
    nc.vector.tensor_scalar_mul(t1, h, 0.5)
    nc.vector.tensor_add(t2, rtf[:, :, 0], t1)
    nc.vector.tensor_mul(t1, dsel[:, :, 0], h)
    nc.vector.tensor_add(cy, t2, t1)
    nc.vector.tensor_scalar_mul(t1, w, 0.5)
    nc.vector.tensor_add(t2, rtf[:, :, 1], t1)
    nc.vector.tensor_mul(t1, dsel[:, :, 1], w)
    nc.vector.tensor_add(cx, t2, t1)
    e = sm.tile([P, NT], F32, tag="e")
    nc.scalar.activation(e, dsel[:, :, 2], mybir.ActivationFunctionType.Exp)
    nc.vector.tensor_mul(h, h, e)
    nc.scalar.activation(e, dsel[:, :, 3], mybir.ActivationFunctionType.Exp)
    nc.vector.tensor_mul(w, w, e)

    ref = sm.tile([P, NT, 4], F32, tag="ref")
    nc.vector.tensor_scalar_mul(t1, h, 0.5)
    nc.vector.tensor_sub(ref[:, :, 0], cy, t1)
    nc.vector.tensor_add(ref[:, :, 2], cy, t1)
    nc.vector.tensor_scalar_mul(t2, w, 0.5)
    nc.vector.tensor_sub(ref[:, :, 1], cx, t2)
    nc.vector.tensor_add(ref[:, :, 3], cx, t2)
    nc.vector.tensor_scalar(
        out=ref, in0=ref, scalar1=0.0, scalar2=1.0,
        op0=mybir.AluOpType.max, op1=mybir.AluOpType.min,
    )

    sc = state["sc"].rearrange("p b n -> p (b n)")
    ob = state["ob"].rearrange("p b n k -> p (b n) k")
    ar = state["ar"].rearrange("p b n -> p (b n)")
    cat = state["cat"].rearrange("p b n k -> p (b n) k")
    negs = state["negs"]

    vf = sm.tile([P, NT], F32, tag="vf")
    nc.vector.tensor_single_scalar(vf, cid, 0.5, op=mybir.AluOpType.is_ge)
    v = sm.tile([P, NT], mybir.dt.uint8, tag="v")
    nc.vector.tensor_mul(v, vf, ge)
    nc.vector.tensor_copy(sc, negs)
    nc.vector.copy_predicated(sc, v, scores)

    nc.vector.scalar_tensor_tensor(
        out=ob, in0=cid.unsqueeze(2).to_broadcast([P, NT, 4]), scalar=2.0,
        in1=ref, op0=mybir.AluOpType.mult, op1=mybir.AluOpType.add,
    )
    ar2 = sm.tile([P, NT, 2], F32, tag="ar2")
    nc.vector.tensor_sub(ar2, ob[:, :, 2:4], ob[:, :, 0:2])
    nc.vector.tensor_mul(ar, ar2[:, :, 0], ar2[:, :, 1])
    nc.vector.tensor_copy(cat[:, :, 0:4], ref)
    nc.vector.tensor_copy(cat[:, :, 4], cid)
    nc.vector.tensor_copy(cat[:, :, 5], scores)


def _nms_image(nc, tc, sm, img, det, state):
    """Cold path per image (see kernel.py)."""
    sc = state["sc"][:, img]
    ob = state["ob"][:, img]
    ar = state["ar"][:, img]
    cat = state["cat"][:, img]
    negs = state["negs"][:, 0:NP]
    mr = state["mr"]

    with tc.For_i(0, K, name=f"nms{img}") as i:
        pm = sm.tile([P, 1], F32, tag=f"pm{img}")
        nc.vector.reduce_max(pm, sc, axis=mybir.AxisListType.X)
        gm = sm.tile([P, 1], F32, tag=f"gm{img}")
        nc.gpsimd.partition_all_reduce(gm, pm, channels=P,
                                       reduce_op=bass_isa.ReduceOp.max)
        msk = sm.tile([P, NP], F32, tag=f"msk{img}")
        nc.vector.tensor_tensor(msk, sc, gm.to_broadcast([P, NP]),
                                op=mybir.AluOpType.is_equal)
        mb6 = sm.tile([P, NP, 6], F32, tag=f"mb6{img}")
        nc.vector.tensor_tensor(
            mb6, cat, msk.unsqueeze(2).to_broadcast([P, NP, 6]),
            op=mybir.AluOpType.mult,
        )
        r6p = sm.tile([P, 6], F32, tag=f"r6p{img}")
        nc.vector.reduce_sum(r6p, mb6.rearrange("p n k -> p k n"),
                             axis=mybir.AxisListType.X)
        r6 = sm.tile([P, 6], F32, tag=f"r6{img}")
        nc.gpsimd.partition_all_reduce(r6, r6p, channels=P,
                                       reduce_op=bass_isa.ReduceOp.add)
        okm = sm.tile([P, 1], F32, tag=f"okm{img}")
        nc.vector.tensor_single_scalar(okm, gm, NEG * 0.5,
                                       op=mybir.AluOpType.is_gt)
        nc.vector.tensor_mul(r6, r6, okm.to_broadcast([P, 6]))
        nc.vector.tensor_copy(det[0:1, bass.ds(img * K * 6 + i * 6, 6)],
                              r6[0:1, :])

        sb = sm.tile([P, 4], F32, tag=f"sb{img}")
        nc.vector.scalar_tensor_tensor(
            out=sb, in0=r6[:, 4:5].to_broadcast([P, 4]), scalar=2.0,
            in1=r6[:, 0:4], op0=mybir.AluOpType.mult, op1=mybir.AluOpType.add,
        )
        mx = sm.tile([P, NP, 2], F32, tag=f"mx{img}")
        nc.vector.tensor_tensor(
            mx, ob[:, :, 0:2], sb[:, 0:2].unsqueeze(1).to_broadcast([P, NP, 2]),
            op=mybir.AluOpType.max,
        )
        mn = sm.tile([P, NP, 2], F32, tag=f"mn{img}")
        nc.vector.tensor_tensor(
            mn, ob[:, :, 2:4], sb[:, 2:4].unsqueeze(1).to_broadcast([P, NP, 2]),
            op=mybir.AluOpType.min,
        )
        nc.vector.tensor_sub(mn, mn, mx)
        nc.vector.tensor_scalar_max(mn, mn, 0.0)
        inter = sm.tile([P, NP], F32, tag=f"inter{img}")
        nc.vector.tensor_mul(inter, mn[:, :, 0], mn[:, :, 1])
        aa2 = sm.tile([P, 2], F32, tag=f"aa2{img}")
        nc.vector.tensor_sub(aa2, sb[:, 2:4], sb[:, 0:2])
        aa = sm.tile([P, 1], F32, tag=f"aa{img}")
        nc.vector.tensor_mul(aa, aa2[:, 0:1], aa2[:, 1:2])
        u = sm.tile([P, NP], F32, tag=f"u{img}")
        nc.vector.scalar_tensor_tensor(
            out=u, in0=ar, scalar=aa[:, 0:1], in1=inter,
            op0=mybir.AluOpType.add, op1=mybir.AluOpType.subtract,
        )
        sup = sm.tile([P, NP], mybir.dt.uint8, tag=f"sup{img}")
        nc.vector.scalar_tensor_tensor(
            out=sup, in0=u, scalar=NMS_T, in1=inter,
            op0=mybir.AluOpType.mult, op1=mybir.AluOpType.is_lt,
        )
        nc.vector.copy_predicated(sc, sup, negs)
        nc.vector.tensor_copy(mr[:, 0:1], gm)
        nc.vector.match_replace(out=sc, in_to_replace=mr, in_values=sc,
                                imm_value=NEG)


def build_nc():
    nc = bacc.Bacc("TRN2", target_bir_lowering=False)
    rois_t = nc.dram_tensor("rois", [BPC, N, 4], F32, kind="ExternalInput")
    probs_t = nc.dram_tensor("probs", [BPC, N, C], F32, kind="ExternalInput")
    deltas_t = nc.dram_tensor("deltas", [BPC, N, C, 4], F32,
                              kind="ExternalInput")
    out_t = nc.dram_tensor("out", [BPC, K, 6], F32, kind="ExternalOutput")
    out_ap = out_t.rearrange("b k s -> (b k s)").unsqueeze(0)

    NDVE = len(_DVE_N)
    NCOL = NDVE + len(_ACT_N) + 1

    # ---- raw SBUF/PSUM state ----
    ptw = nc.alloc_sbuf_tensor("ptw_sb", [P, BPC, NP, C], F32).ap()
    det = nc.alloc_sbuf_tensor("det_sb", [1, BPC * K * 6], F32).ap()
    cnt = nc.alloc_sbuf_tensor("cnt_sb", [P, NCOL], F32).ap()
    scrA = nc.alloc_sbuf_tensor("scrA_sb", [P, max(_DVE_N), C],
                                mybir.dt.uint8).ap()
    scrB = nc.alloc_sbuf_tensor("scrB_sb", [P, max(_ACT_N), C],
                                mybir.dt.bfloat16).ap()
    biasT = nc.alloc_sbuf_tensor("biasT_sb", [P, 1], F32).ap()
    ones = nc.alloc_sbuf_tensor("ones_sb", [P, 1], F32).ap()
    gi = nc.alloc_sbuf_tensor("gi_sb", [1, 1], I32).ap()
    csum = nc.alloc_psum_tensor("csum_ps", [1, NCOL], F32).ap()

    s_chunk = [nc.alloc_semaphore(f"s_ch{i}") for i in range(len(CHUNKS))]
    s_out = nc.alloc_semaphore("s_out")
    s_det = nc.alloc_semaphore("s_det")
    s_pre = nc.alloc_semaphore("s_pre")
    s_done = nc.alloc_semaphore("s_done")
    s_gi = nc.alloc_semaphore("s_gi")
    s_mm = nc.alloc_semaphore("s_mm")
    accdrain = nc.alloc_sbuf_tensor("accdrain_sb", [P, 1], F32).ap()

    # Defensive clears: semaphore start-of-kernel state is only zero by
    # convention (the previous NEFF's epilogue); clear each semaphore on
    # the engine that first increments it, before any use, so a stale
    # value can never release a wait early.
    chn = [s.num for s in s_chunk]
    assert chn == list(range(chn[0], chn[0] + len(chn))), chn
    nc.sync.sem_clear(range(chn[0], chn[-1] + 1))
    nc.scalar.sem_clear(s_out)
    nc.gpsimd.sem_clear(s_det)
    assert s_gi.num == s_pre.num + 2, (s_pre.num, s_done.num, s_gi.num)
    nc.vector.sem_clear(range(s_pre.num, s_gi.num + 1))
    nc.tensor.sem_clear(s_mm)

    psrc = [probs_t[b].rearrange("(p n) c -> p n c", p=P) for b in range(BPC)]

    # ---- probs stream: desc-gens are the very first body instructions ----
    for i, (b, lo, hi, eng) in enumerate(CHUNKS):
        nc.sync.dma_start(out=ptw[:, b, lo:hi],
                          in_=psrc[b][:, lo:hi]).then_inc(s_chunk[i], 16)

    # zeros-out up front on the (low-priority) Act queue
    nc.gpsimd.memset(det, 0.0).then_inc(s_det, 1)
    nc.scalar.wait_ge(s_det, 1)
    nc.scalar.dma_start(out=out_ap, in_=det[0:1]).then_inc(s_out, 16)

    # constants, on otherwise-idle engines during the DMA window
    nc.vector.memset(cnt[:, NCOL - 1:NCOL], float(C * sum(_ACT_N)))
    nc.vector.memset(ones, 1.0)
    nc.vector.memset(biasT, -MIN_CONF).then_inc(s_pre, 1)
    nc.scalar.wait_ge(s_pre, 1)  # bias ready before first ACTIVATE
    # drain the ACT accumulator: its start-of-kernel value is device state,
    # so snapshot-and-reset it into a scratch slot before the real sums
    nc.scalar.activation(accdrain, biasT, mybir.ActivationFunctionType.Sign,
                         bias=biasT[:, 0:1], accum_out=accdrain)

    # ---- per-chunk gate ----
    dve_idx = [i for i, c in enumerate(CHUNKS) if c[3] == 0]
    act_idx = [i for i, c in enumerate(CHUNKS) if c[3] == 1]
    col_dve, col_act = 0, NDVE
    for i, (b, lo, hi, eng) in enumerate(CHUNKS):
        n = hi - lo
        if eng == 0:
            # per-chunk count of elements >= MIN_CONF via is_ge +
            # accumulator; the column is a nonneg integer that feeds the
            # ones-matmul directly (no conversion op on the critical path,
            # and the bass-managed op+accumulator-read pair has no
            # same-engine read-after-write hazard)
            nc.vector.wait_ge(s_chunk[i], 16)
            op = nc.vector.tensor_scalar(
                out=scrA[:, 0:n], in0=ptw[:, b, lo:hi],
                scalar1=MIN_CONF, scalar2=None,
                op0=mybir.AluOpType.is_ge, op1=mybir.AluOpType.add,
                accum_out=cnt[:, col_dve:col_dve + 1],
            )
            col_dve += 1
            if i == dve_idx[-1]:
                op.then_inc(s_done, 1)
        else:
            nc.scalar.wait_ge(s_chunk[i], 16)
            op = nc.scalar.activation(
                scrB[:, 0:n], ptw[:, b, lo:hi],
                mybir.ActivationFunctionType.Sign,
                bias=biasT[:, 0:1],
                accum_out=cnt[:, col_act:col_act + 1],
            )
            col_act += 1
            if i == act_idx[-1]:
                op.then_inc(s_done, 1)

    # ---- combine: ones^T @ cnt -> csum, reduce -> gi ----
    nc.tensor.wait_ge(s_done, 2)
    nc.tensor.matmul(csum, ones, cnt, start=True, stop=True).then_inc(s_mm, 1)
    nc.vector.wait_ge(s_mm, 1)
    with nc.allow_low_precision(reason="exact small-int sum, int32 output"):
        nc.vector.reduce_sum(gi, csum,
                             axis=mybir.AxisListType.X).then_inc(s_gi, 1)

    # make sure the zeros-out DMA has retired before the program can end
    nc.gpsimd.wait_ge(s_out, 16)

    for eng in (nc.sync, nc.vector, nc.scalar, nc.gpsimd, nc.tensor):
        eng.wait_ge(s_gi, 1)
    gv = nc.values_load(gi[0:1, 0:1], min_val=0, max_val=2 * BPC * N * C,
                        skip_runtime_bounds_check=True)

    # ---- guarded cold path: TileContext nested in the raw If ----
    if not NOGUARD:
        with nc.If(gv >= 1):
            with TileContext(nc) as tc:
                with (
                    tc.tile_pool(name="big", bufs=1) as big,
                    tc.tile_pool(name="small", bufs=1) as sm,
                ):
                    NT = BPC * NP
                    crev = sm.tile([P, NT, C], F32, tag="crev")
                    nc.gpsimd.iota(crev, pattern=[[0, NT], [-1, C]],
                                   base=C - 1, channel_multiplier=0,
                                   allow_small_or_imprecise_dtypes=True)
                    negs = sm.tile([P, NT], F32, tag="negs")
                    nc.gpsimd.memset(negs, NEG)
                    mr = sm.tile([P, 8], F32, tag="mr")
                    nc.gpsimd.memset(mr, NEG)

                    sc_w = sm.tile([P, BPC, NP], F32, tag="sc")
                    ob_w = sm.tile([P, BPC, NP, 4], F32, tag="ob")
                    ar_w = sm.tile([P, BPC, NP], F32, tag="ar")
                    cat_w = sm.tile([P, BPC, NP, 6], F32, tag="cat")
                    state = {
                        "negs": negs,
                        "mr": mr,
                        "sc": sc_w,
                        "ob": ob_w,
                        "ar": ar_w,
                        "cat": cat_w,
                    }
                    scw = sm.tile([P, BPC, NP], F32, tag="scores")
                    det2 = sm.tile([1, BPC * K * 6], F32, tag="det2")

                    rt = sm.tile([P, BPC, NP, 4], F32, tag="rois")
                    dt_ = big.tile([P, BPC, NP, C, 4], F32, tag="deltas")
                    for img in range(BPC):
                        nc.sync.dma_start(
                            out=rt[:, img],
                            in_=rois_t[img].rearrange("(p n) k -> p n k",
                                                      p=P))
                        dsrc = deltas_t[img].rearrange("(p n) c k -> p n c k",
                                                       p=P)
                        for s in range(8):
                            sl = slice(16 * s, 16 * s + 16)
                            nc.sync.dma_start(out=dt_[sl, img], in_=dsrc[sl])
                    ptw_v = ptw.rearrange("p b n c -> p (b n) c")
                    _refine_twin(nc, tc, sm, ptw_v, scw, rt, dt_, crev, state)
                    for img in range(BPC):
                        _nms_image(nc, tc, sm, img, det2, state)
                    nc.sync.dma_start(out=out_ap, in_=det2[0:1])
        nc.end_ifs()

    nc.compile()
    return nc


LAST_RESULTS = None


def kernel(rois, probs, deltas):
    global LAST_RESULTS
    from concourse import bass_utils

    nc = build_nc()
    in_maps = []
    for c in range(NCORES):
        sl = slice(c * BPC, (c + 1) * BPC)
        in_maps.append({
            "rois": np.ascontiguousarray(rois[sl], dtype=np.float32),
            "probs": np.ascontiguousarray(probs[sl], dtype=np.float32),
            "deltas": np.ascontiguousarray(deltas[sl], dtype=np.float32),
        })
    res = bass_utils.run_bass_kernel_spmd(nc, in_maps,
                                          core_ids=list(range(NCORES)))
    LAST_RESULTS = res
    return np.concatenate([r["out"] for r in res.results], axis=0)


# revision 21
# speedup vs baseline: 1.1062x; 1.1062x over previous
"""Trainium2 Bass kernel for DetectionLayer (refine + per-class NMS).

Contract: kernel(rois, probs, deltas) with FULL inputs
  rois   [16, 4096, 4]   f32
  probs  [16, 4096, 81]  f32
  deltas [16, 4096, 81, 4] f32
returns [16, 100, 6] f32 detections, matching the jax reference.

Sharding: pure data parallel - 2 images per core across 8 NeuronCores.

The always-taken fast path (probs stream + confidence gate + early exit)
is written in raw Bass with explicit semaphores so it starts before and
ends without TileContext's entry/exit handshakes: probs stream over the
Sync HWDGE queue in 7 decreasing chunks at ~420GB/s, DVE is_ge+count and
ACT sign+accumulate gate alternate chunks as they land, a PE ones-matmul
folds partitions, and one int32 reduce feeds the branch.  The all-zeros
output is DMA'd to HBM up front (hidden under the probs window), so when
no prob reaches min-confidence the kernel ends right after the branch.
The cold path (refine + per-class NMS, only when any prob >= 0.7) lives
in a TileContext nested inside the raw nc.If body, so its scheduling
cost is paid only when detections exist.
"""

import os as _os

import numpy as np

import concourse.bacc as bacc
import concourse.bass as bass
import concourse.bass_isa as bass_isa
import concourse.mybir as mybir
from concourse.tile import TileContext

B = 16
NCORES = 8
BPC = B // NCORES
N = 4096
C = 81
K = 100
P = 128
NP = N // P
NEG = -1e9
MIN_CONF = 0.7
NMS_T = 0.3
F32 = mybir.dt.float32
I32 = mybir.dt.int32

NOGUARD = _os.environ.get("DETK_NOGUARD", "0") == "1"

# (img, lo, hi, eng): eng 0 = DVE is_ge, 1 = ACT sign.  All on the Sync
# HWDGE queue (the Act queue is deprioritized by the DMA engines), sizes
# decreasing, engines alternating, last chunk tiny + DVE.
CHUNKS = [
    (0, 0, 18, 0),
    (0, 18, 32, 1),
    (1, 0, 11, 0),
    (1, 11, 19, 1),
    (1, 19, 27, 0),
    (1, 27, 30, 1),
    (1, 30, 32, 0),
]
if _os.environ.get("DETK_CHUNKS"):
    import json as _json
    CHUNKS = [tuple(c) for c in _json.loads(_os.environ["DETK_CHUNKS"])]

_DVE_N = [hi - lo for (_, lo, hi, e) in CHUNKS if e == 0]
_ACT_N = [hi - lo for (_, lo, hi, e) in CHUNKS if e == 1]


def _refine_twin(nc, tc, sm, ptw, scw, rt, dt_, crev, state):
    """Cold path, both images at once (see kernel.py)."""
    NT = BPC * NP
    pt = ptw
    scores = scw.rearrange("p b n -> p (b n)")
    rtf = rt.rearrange("p b n k -> p (b n) k")

    nc.vector.reduce_max(scores, pt, axis=mybir.AxisListType.X)
    ge = sm.tile([P, NT], F32, tag="ge")
    nc.vector.tensor_single_scalar(ge, scores, MIN_CONF,
                                   op=mybir.AluOpType.is_ge)

    m = pt
    nc.vector.tensor_tensor(
        m, pt, scores.unsqueeze(2).to_broadcast([P, NT, C]),
        op=mybir.AluOpType.is_equal,
    )

    d_perm = dt_.rearrange("p b n c k -> p (b n) k c")
    nc.vector.tensor_tensor(
        d_perm, d_perm, m.unsqueeze(2).to_broadcast([P, NT, 4, C]),
        op=mybir.AluOpType.mult,
    )
    dsel = sm.tile([P, NT, 4], F32, tag="dsel")
    nc.vector.reduce_sum(dsel, d_perm, axis=mybir.AxisListType.X)

    nc.vector.tensor_tensor(m, m, crev, op=mybir.AluOpType.mult)
    cid = sm.tile([P, NT], F32, tag="cid")
    nc.vector.reduce_max(cid, m, axis=mybir.AxisListType.X)
    nc.vector.tensor_scalar(
        out=cid, in0=cid, scalar1=-1.0, scalar2=float(C - 1),
        op0=mybir.AluOpType.mult, op1=mybir.AluOpType.add,
    )

    nc.vector.tensor_scalar_mul(dsel[:, :, 0:2], dsel[:, :, 0:2], 0.1)
    nc.vector.tensor_scalar_mul(dsel[:, :, 2:4], dsel[:, :, 2:4], 0.2)

    h = sm.tile([P, NT], F32, tag="h")
    w = sm.tile([P, NT], F32, tag="w")
    nc.vector.tensor_sub(h, rtf[:, :, 2], rtf[:, :, 0])
    nc.vector.tensor_sub(w, rtf[:, :, 3], rtf[:, :, 1])
    t1 = sm.tile([P, NT], F32, tag="t1")
    t2 = sm.tile([P, NT], F32, tag="t2")
    cy = sm.tile([P, NT], F32, tag="cy")
    cx = sm.tile([P, NT], F32, tag="cx")
    nc.vector.tensor_scalar_mul(t1, h, 0.5)
    nc.vector.tensor_add(t2, rtf[:, :, 0], t1)
    nc.vector.tensor_mul(t1, dsel[:, :, 0], h)
    nc.vector.tensor_add(cy, t2, t1)
    nc.vector.tensor_scalar_mul(t1, w, 0.5)
    nc.vector.tensor_add(t2, rtf[:, :, 1], t1)
    nc.vector.tensor_mul(t1, dsel[:, :, 1], w)
    nc.vector.tensor_add(cx, t2, t1)
    e = sm.tile([P, NT], F32, tag="e")
    nc.scalar.activation(e, dsel[:, :, 2], mybir.ActivationFunctionType.Exp)
    nc.vector.tensor_mul(h, h, e)
    nc.scalar.activation(e, dsel[:, :, 3], mybir.ActivationFunctionType.Exp)
    nc.vector.tensor_mul(w, w, e)

    ref = sm.tile([P, NT, 4], F32, tag="ref")
    nc.vector.tensor_scalar_mul(t1, h, 0.5)
    nc.vector.tensor_sub(ref[:, :, 0], cy, t1)
    nc.vector.tensor_add(ref[:, :, 2], cy, t1)
    nc.vector.tensor_scalar_mul(t2, w, 0.5)
    nc.vector.tensor_sub(ref[:, :, 1], cx, t2)
    nc.vector.tensor_add(ref[:, :, 3], cx, t2)
    nc.vector.tensor_scalar(
        out=ref, in0=ref, scalar1=0.0, scalar2=1.0,
        op0=mybir.AluOpType.max, op1=mybir.AluOpType.min,
    )

    sc = state["sc"].rearrange("p b n -> p (b n)")
    ob = state["ob"].rearrange("p b n k -> p (b n) k")
    ar = state["ar"].rearrange("p b n -> p (b n)")
    cat = state["cat"].rearrange("p b n k -> p (b n) k")
    negs = state["negs"]

    vf = sm.tile([P, NT], F32, tag="vf")
    nc.vector.tensor_single_scalar(vf, cid, 0.5, op=mybir.AluOpType.is_ge)
    v = sm.tile([P, NT], mybir.dt.uint8, tag="v")
    nc.vector.tensor_mul(v, vf, ge)
    nc.vector.tensor_copy(sc, negs)
    nc.vector.copy_predicated(sc, v, scores)

    nc.vector.scalar_tensor_tensor(
        out=ob, in0=cid.unsqueeze(2).to_broadcast([P, NT, 4]), scalar=2.0,
        in1=ref, op0=mybir.AluOpType.mult, op1=mybir.AluOpType.add,
    )
    ar2 = sm.tile([P, NT, 2], F32, tag="ar2")
    nc.vector.tensor_sub(ar2, ob[:, :, 2:4], ob[:, :, 0:2])
    nc.vector.tensor_mul(ar, ar2[:, :, 0], ar2[:, :, 1])
    nc.vector.tensor_copy(cat[:, :, 0:4], ref)
    nc.vector.tensor_copy(cat[:, :, 4], cid)
    nc.vector.tensor_copy(cat[:, :, 5], scores)


def _nms_image(nc, tc, sm, img, det, state):
    """Cold path per image (see kernel.py)."""
    sc = state["sc"][:, img]
    ob = state["ob"][:, img]
    ar = state["ar"][:, img]
    cat = state["cat"][:, img]
    negs = state["negs"][:, 0:NP]
    mr = state["mr"]

    with tc.For_i(0, K, name=f"nms{img}") as i:
        pm = sm.tile([P, 1], F32, tag=f"pm{img}")
        nc.vector.reduce_max(pm, sc, axis=mybir.AxisListType.X)
        gm = sm.tile([P, 1], F32, tag=f"gm{img}")
        nc.gpsimd.partition_all_reduce(gm, pm, channels=P,
                                       reduce_op=bass_isa.ReduceOp.max)
        msk = sm.tile([P, NP], F32, tag=f"msk{img}")
        nc.vector.tensor_tensor(msk, sc, gm.to_broadcast([P, NP]),
                                op=mybir.AluOpType.is_equal)
        mb6 = sm.tile([P, NP, 6], F32, tag=f"mb6{img}")
        nc.vector.tensor_tensor(
            mb6, cat, msk.unsqueeze(2).to_broadcast([P, NP, 6]),
            op=mybir.AluOpType.mult,
        )
        r6p = sm.tile([P, 6], F32, tag=f"r6p{img}")
        nc.vector.reduce_sum(r6p, mb6.rearrange("p n k -> p k n"),
                             axis=mybir.AxisListType.X)
        r6 = sm.tile([P, 6], F32, tag=f"r6{img}")
        nc.gpsimd.partition_all_reduce(r6, r6p, channels=P,
                                       reduce_op=bass_isa.ReduceOp.add)
        okm = sm.tile([P, 1], F32, tag=f"okm{img}")
        nc.vector.tensor_single_scalar(okm, gm, NEG * 0.5,
                                       op=mybir.AluOpType.is_gt)
        nc.vector.tensor_mul(r6, r6, okm.to_broadcast([P, 6]))
        nc.vector.tensor_copy(det[0:1, bass.ds(img * K * 6 + i * 6, 6)],
                              r6[0:1, :])

        sb = sm.tile([P, 4], F32, tag=f"sb{img}")
        nc.vector.scalar_tensor_tensor(
            out=sb, in0=r6[:, 4:5].to_broadcast([P, 4]), scalar=2.0,
            in1=r6[:, 0:4], op0=mybir.AluOpType.mult, op1=mybir.AluOpType.add,
        )
        mx = sm.tile([P, NP, 2], F32, tag=f"mx{img}")
        nc.vector.tensor_tensor(
            mx, ob[:, :, 0:2], sb[:, 0:2].unsqueeze(1).to_broadcast([P, NP, 2]),
            op=mybir.AluOpType.max,
        )
        mn = sm.tile([P, NP, 2], F32, tag=f"mn{img}")
        nc.vector.tensor_tensor(
            mn, ob[:, :, 2:4], sb[:, 2:4].unsqueeze(1).to_broadcast([P, NP, 2]),
            op=mybir.AluOpType.min,
        )
        nc.vector.tensor_sub(mn, mn, mx)
        nc.vector.tensor_scalar_max(mn, mn, 0.0)
        inter = sm.tile([P, NP], F32, tag=f"inter{img}")
        nc.vector.tensor_mul(inter, mn[:, :, 0], mn[:, :, 1])
        aa2 = sm.tile([P, 2], F32, tag=f"aa2{img}")
        nc.vector.tensor_sub(aa2, sb[:, 2:4], sb[:, 0:2])
        aa = sm.tile([P, 1], F32, tag=f"aa{img}")
        nc.vector.tensor_mul(aa, aa2[:, 0:1], aa2[:, 1:2])
        u = sm.tile([P, NP], F32, tag=f"u{img}")
        nc.vector.scalar_tensor_tensor(
            out=u, in0=ar, scalar=aa[:, 0:1], in1=inter,
            op0=mybir.AluOpType.add, op1=mybir.AluOpType.subtract,
        )
        sup = sm.tile([P, NP], mybir.dt.uint8, tag=f"sup{img}")
        nc.vector.scalar_tensor_tensor(
            out=sup, in0=u, scalar=NMS_T, in1=inter,
            op0=mybir.AluOpType.mult, op1=mybir.AluOpType.is_lt,
        )
        nc.vector.copy_predicated(sc, sup, negs)
        nc.vector.tensor_copy(mr[:, 0:1], gm)
        nc.vector.match_replace(out=sc, in_to_replace=mr, in_values=sc,
                                imm_value=NEG)


def build_nc():
    nc = bacc.Bacc("TRN2", target_bir_lowering=False)
    rois_t = nc.dram_tensor("rois", [BPC, N, 4], F32, kind="ExternalInput")
    probs_t = nc.dram_tensor("probs", [BPC, N, C], F32, kind="ExternalInput")
    deltas_t = nc.dram_tensor("deltas", [BPC, N, C, 4], F32,
                              kind="ExternalInput")
    out_t = nc.dram_tensor("out", [BPC, K, 6], F32, kind="ExternalOutput")
    out_ap = out_t.rearrange("b k s -> (b k s)").unsqueeze(0)

    NDVE = len(_DVE_N)
    NCOL = NDVE + len(_ACT_N) + 1

    # ---- raw SBUF/PSUM state ----
    ptw = nc.alloc_sbuf_tensor("ptw_sb", [P, BPC, NP, C], F32).ap()
    det = nc.alloc_sbuf_tensor("det_sb", [1, BPC * K * 6], F32).ap()
    cnt = nc.alloc_sbuf_tensor("cnt_sb", [P, NCOL], F32).ap()
    scrA = nc.alloc_sbuf_tensor("scrA_sb", [P, max(_DVE_N), C],
                                mybir.dt.uint8).ap()
    scrB = nc.alloc_sbuf_tensor("scrB_sb", [P, max(_ACT_N), C],
                                mybir.dt.bfloat16).ap()
    biasT = nc.alloc_sbuf_tensor("biasT_sb", [P, 1], F32).ap()
    ones = nc.alloc_sbuf_tensor("ones_sb", [P, 1], F32).ap()
    gi = nc.alloc_sbuf_tensor("gi_sb", [1, 1], I32).ap()
    csum = nc.alloc_psum_tensor("csum_ps", [1, NCOL], F32).ap()

    s_chunk = [nc.alloc_semaphore(f"s_ch{i}") for i in range(len(CHUNKS))]
    s_out = nc.alloc_semaphore("s_out")
    s_det = nc.alloc_semaphore("s_det")
    s_pre = nc.alloc_semaphore("s_pre")
    s_done = nc.alloc_semaphore("s_done")
    s_gi = nc.alloc_semaphore("s_gi")
    s_mm = nc.alloc_semaphore("s_mm")
    accdrain = nc.alloc_sbuf_tensor("accdrain_sb", [P, 1], F32).ap()

    # Defensive clears: semaphore start-of-kernel state is only zero by
    # convention (the previous NEFF's epilogue); clear each semaphore on
    # the engine that first increments it, before any use, so a stale
    # value can never release a wait early.
    chn = [s.num for s in s_chunk]
    assert chn == list(range(chn[0], chn[0] + len(chn))), chn
    assert s_out.num == chn[-1] + 1, (chn, s_out.num)
    nc.sync.sem_clear(range(chn[0], s_out.num + 1))
    nc.gpsimd.sem_clear(s_det)
    assert s_gi.num == s_pre.num + 2, (s_pre.num, s_done.num, s_gi.num)
    nc.vector.sem_clear(range(s_pre.num, s_gi.num + 1))
    nc.tensor.sem_clear(s_mm)

    psrc = [probs_t[b].rearrange("(p n) c -> p n c", p=P) for b in range(BPC)]

    # ---- probs stream: desc-gens are the very first body instructions ----
    for i, (b, lo, hi, eng) in enumerate(CHUNKS):
        nc.sync.dma_start(out=ptw[:, b, lo:hi],
                          in_=psrc[b][:, lo:hi]).then_inc(s_chunk[i], 16)

    # zeros-out DMA queued on Sync AFTER all probs chunks: a second queue's
    # transfer mid-stream measurably dips the probs bandwidth, and Q1-last
    # means the 4.8KB lands right after the final chunk (~17us), still well
    # before the branch-value loads and the exit drain need it
    nc.gpsimd.memset(det, 0.0).then_inc(s_det, 1)
    nc.sync.wait_ge(s_det, 1)
    nc.sync.dma_start(out=out_ap, in_=det[0:1]).then_inc(s_out, 16)

    # constants, on otherwise-idle engines during the DMA window
    nc.vector.memset(cnt[:, NCOL - 1:NCOL], float(C * sum(_ACT_N)))
    nc.vector.memset(ones, 1.0)
    nc.vector.memset(biasT, -MIN_CONF).then_inc(s_pre, 1)
    nc.scalar.wait_ge(s_pre, 1)  # bias ready before first ACTIVATE
    # drain the ACT accumulator: its start-of-kernel value is device state,
    # so snapshot-and-reset it into a scratch slot before the real sums
    nc.scalar.activation(accdrain, biasT, mybir.ActivationFunctionType.Sign,
                         bias=biasT[:, 0:1], accum_out=accdrain)

    # ---- per-chunk gate ----
    dve_idx = [i for i, c in enumerate(CHUNKS) if c[3] == 0]
    act_idx = [i for i, c in enumerate(CHUNKS) if c[3] == 1]
    col_dve, col_act = 0, NDVE
    for i, (b, lo, hi, eng) in enumerate(CHUNKS):
        n = hi - lo
        if eng == 0:
            # per-chunk count of elements >= MIN_CONF via is_ge +
            # accumulator; the column is a nonneg integer that feeds the
            # ones-matmul directly (no conversion op on the critical path,
            # and the bass-managed op+accumulator-read pair has no
            # same-engine read-after-write hazard)
            nc.vector.wait_ge(s_chunk[i], 16)
            op = nc.vector.tensor_scalar(
                out=scrA[:, 0:n], in0=ptw[:, b, lo:hi],
                scalar1=MIN_CONF, scalar2=None,
                op0=mybir.AluOpType.is_ge, op1=mybir.AluOpType.add,
                accum_out=cnt[:, col_dve:col_dve + 1],
            )
            col_dve += 1
            if i == dve_idx[-1]:
                op.then_inc(s_done, 1)
        else:
            nc.scalar.wait_ge(s_chunk[i], 16)
            op = nc.scalar.activation(
                scrB[:, 0:n], ptw[:, b, lo:hi],
                mybir.ActivationFunctionType.Sign,
                bias=biasT[:, 0:1],
                accum_out=cnt[:, col_act:col_act + 1],
            )
            col_act += 1
            if i == act_idx[-1]:
                op.then_inc(s_done, 1)

    # ---- combine: ones^T @ cnt -> csum, reduce -> gi ----
    nc.tensor.wait_ge(s_done, 2)
    nc.tensor.matmul(csum, ones, cnt, start=True, stop=True).then_inc(s_mm, 1)
    nc.vector.wait_ge(s_mm, 1)
    with nc.allow_low_precision(reason="exact small-int sum, int32 output"):
        nc.vector.reduce_sum(gi, csum,
                             axis=mybir.AxisListType.X).then_inc(s_gi, 1)

    # make sure the zeros-out DMA has retired before the program can end
    nc.gpsimd.wait_ge(s_out, 16)

    for eng in (nc.sync, nc.vector, nc.scalar, nc.gpsimd, nc.tensor):
        eng.wait_ge(s_gi, 1)
    gv = nc.values_load(gi[0:1, 0:1], min_val=0, max_val=2 * BPC * N * C,
                        skip_runtime_bounds_check=True)

    # ---- guarded cold path: TileContext nested in the raw If ----
    if not NOGUARD:
        with nc.If(gv >= 1):
            with TileContext(nc) as tc:
                with (
                    tc.tile_pool(name="big", bufs=1) as big,
                    tc.tile_pool(name="small", bufs=1) as sm,
                ):
                    NT = BPC * NP
                    crev = sm.tile([P, NT, C], F32, tag="crev")
                    nc.gpsimd.iota(crev, pattern=[[0, NT], [-1, C]],
                                   base=C - 1, channel_multiplier=0,
                                   allow_small_or_imprecise_dtypes=True)
                    negs = sm.tile([P, NT], F32, tag="negs")
                    nc.gpsimd.memset(negs, NEG)
                    mr = sm.tile([P, 8], F32, tag="mr")
                    nc.gpsimd.memset(mr, NEG)

                    sc_w = sm.tile([P, BPC, NP], F32, tag="sc")
                    ob_w = sm.tile([P, BPC, NP, 4], F32, tag="ob")
                    ar_w = sm.tile([P, BPC, NP], F32, tag="ar")
                    cat_w = sm.tile([P, BPC, NP, 6], F32, tag="cat")
                    state = {
                        "negs": negs,
                        "mr": mr,
                        "sc": sc_w,
                        "ob": ob_w,
                        "ar": ar_w,
                        "cat": cat_w,
                    }
                    scw = sm.tile([P, BPC, NP], F32, tag="scores")
                    det2 = sm.tile([1, BPC * K * 6], F32, tag="det2")

                    rt = sm.tile([P, BPC, NP, 4], F32, tag="rois")
                    dt_ = big.tile([P, BPC, NP, C, 4], F32, tag="deltas")
                    for img in range(BPC):
                        nc.sync.dma_start(
                            out=rt[:, img],
                            in_=rois_t[img].rearrange("(p n) k -> p n k",
                                                      p=P))
                        dsrc = deltas_t[img].rearrange("(p n) c k -> p n c k",
                                                       p=P)
                        for s in range(8):
                            sl = slice(16 * s, 16 * s + 16)
                            nc.sync.dma_start(out=dt_[sl, img], in_=dsrc[sl])
                    ptw_v = ptw.rearrange("p b n c -> p (b n) c")
                    _refine_twin(nc, tc, sm, ptw_v, scw, rt, dt_, crev, state)
                    for img in range(BPC):
                        _nms_image(nc, tc, sm, img, det2, state)
                    nc.sync.dma_start(out=out_ap, in_=det2[0:1])
        nc.end_ifs()

    nc.compile()
    return nc


LAST_RESULTS = None


def kernel(rois, probs, deltas):
    global LAST_RESULTS
    from concourse import bass_utils

    nc = build_nc()
    in_maps = []
    for c in range(NCORES):
        sl = slice(c * BPC, (c + 1) * BPC)
        in_maps.append({
            "rois": np.ascontiguousarray(rois[sl], dtype=np.float32),
            "probs": np.ascontiguousarray(probs[sl], dtype=np.float32),
            "deltas": np.ascontiguousarray(deltas[sl], dtype=np.float32),
        })
    res = bass_utils.run_bass_kernel_spmd(nc, in_maps,
                                          core_ids=list(range(NCORES)))
    LAST_RESULTS = res
    return np.concatenate([r["out"] for r in res.results], axis=0)
